# revision 8
# baseline (speedup 1.0000x reference)
"""Trainium2 Bass kernel for nn_AttentionHead.

Computation (per batch b):
    Q = Wq @ x_b, K = Wk @ x_b, V = Wv @ x_b        (x_b: [C=256, N=4096])
    S = Q^T K   [N, N];  A = softmax_k(S)
    out_b = V @ A^T                                  ([VC=128, N])

Sharding: 8 cores = 4 batches x 2 query-halves. Each core computes K/V for
its full batch and Q for its 2048-query half. Inside, a flash-style loop
over 32 key chunks of 128 never materializes the full [4096, 4096] affinity.
The softmax denominators come for free from a ones-column appended to the
PV stationary operand; normalization happens on the host during unshard.
"""

import numpy as np

B, C, VC, H, W = 4, 256, 128, 64, 64
N = H * W            # keys per batch
MQ = N // 2          # queries per core
QT = 1024            # query tile (PSUM-sized)
KC = N // 128        # key chunks of 128
BW = 132             # V^T block width: [64 | 64 | 1 ones | 3 pad] (16B-aligned slices)

_cached_nc = None


def _build():
    from contextlib import ExitStack

    import concourse.bacc as bacc
    import concourse.mybir as mybir
    import concourse.tile as tile

    f32 = mybir.dt.float32
    f32r = mybir.dt.float32r
    Exp = mybir.ActivationFunctionType.Exp

    nc = bacc.Bacc("TRN2", target_bir_lowering=False, debug=False, num_devices=8)

    xk_d = nc.dram_tensor("xk", [C, N], f32r, kind="ExternalInput")
    xq_d = nc.dram_tensor("xq", [C, MQ], f32r, kind="ExternalInput")
    w_d = {
        "wq": nc.dram_tensor("wq", [C, VC], f32r, kind="ExternalInput"),
        "wk": nc.dram_tensor("wk", [C, VC], f32r, kind="ExternalInput"),
        "wv": nc.dram_tensor("wv", [C, VC], f32r, kind="ExternalInput"),
    }
    id_d = nc.dram_tensor("ident", [128, 128], f32, kind="ExternalInput")
    ones_d = nc.dram_tensor("ones", [128, KC], f32r, kind="ExternalInput")
    oa_d = nc.dram_tensor("oa", [2, 65, QT], f32, kind="ExternalOutput")
    ob_d = nc.dram_tensor("ob", [2, 64, QT], f32, kind="ExternalOutput")

    with tile.TileContext(nc) as tc, ExitStack() as ctx:
        persist = ctx.enter_context(tc.tile_pool(name="persist", bufs=1))
        wpool = ctx.enter_context(tc.tile_pool(name="w", bufs=1))

        wts = {}
        for nm in ("wq", "wk", "wv"):
            for cc in range(2):
                t = wpool.tile([128, VC], f32r, tag=f"{nm}{cc}")
                nc.sync.dma_start(t[:], w_d[nm][cc * 128 : (cc + 1) * 128, :])
                wts[(nm, cc)] = t
        ident = wpool.tile([128, 128], f32, tag="ident")
        nc.sync.dma_start(ident[:], id_d[:, :])

        K_t = persist.tile([128, N], f32r, tag="K")
        Q_t = persist.tile([128, MQ], f32r, tag="Q")
        VT = persist.tile([128, KC * BW], f32r, tag="VT")

        with (
            tc.tile_pool(name="xp", bufs=1) as xp,
            tc.tile_pool(name="vp", bufs=1) as vp,
            tc.tile_pool(name="projps", bufs=4, space="PSUM") as pps,
            tc.tile_pool(name="trps", bufs=4, space="PSUM") as tps,
        ):
            xk_t = []
            xq_t = []
            for cc in range(2):
                xkt = xp.tile([128, N], f32r, tag=f"xk{cc}")
                for h in range(2):
                    nc.sync.dma_start(
                        xkt[:, h * 2048 : (h + 1) * 2048],
                        xk_d[cc * 128 : (cc + 1) * 128, h * 2048 : (h + 1) * 2048],
                    )
                xk_t.append(xkt)
                xqt = xp.tile([128, MQ], f32r, tag=f"xq{cc}")
                nc.sync.dma_start(xqt[:], xq_d[cc * 128 : (cc + 1) * 128, :])
                xq_t.append(xqt)
            V_t = vp.tile([128, N], f32, tag="V")

            def proj(dst, wnm, xt, ncols):
                for t in range(ncols // 512):
                    ps = pps.tile([128, 512], f32, tag="projps")
                    for cc in range(2):
                        nc.tensor.matmul(
                            ps[:],
                            wts[(wnm, cc)][:],
                            xt[cc][:, t * 512 : (t + 1) * 512],
                            start=(cc == 0),
                            stop=(cc == 1),
                        )
                    nc.vector.tensor_copy(dst[:, t * 512 : (t + 1) * 512], ps[:])

            proj(K_t, "wk", xk_t, N)
            proj(Q_t, "wq", xq_t, MQ)
            proj(V_t, "wv", xk_t, N)

            # ones columns (slot 128 of each BW-block) for the softmax sums
            ones_view = VT[:].rearrange("p (j c) -> p j c", c=BW)[:, :, 128:129]
            nc.sync.dma_start(ones_view, ones_d[:, :])

            # V^T blocks: slots [0:64] = V^T[:, 64:128], slots [64:128] = V^T[:, 0:64]
            for j in range(KC):
                tp = tps.tile([128, 128], f32, tag="trps")
                nc.tensor.transpose(tp[:], V_t[:, j * 128 : (j + 1) * 128], ident[:])
                nc.vector.tensor_copy(VT[:, j * BW + 64 : j * BW + 128], tp[:, 0:64])
                nc.vector.tensor_copy(VT[:, j * BW : j * BW + 64], tp[:, 64:128])

        with (
            tc.tile_pool(name="spool", bufs=2, space="PSUM") as spool,
            tc.tile_pool(name="papool", bufs=1, space="PSUM") as papool,
            tc.tile_pool(name="pbpool", bufs=1, space="PSUM") as pbpool,
            tc.tile_pool(name="epool", bufs=3) as epool,
            tc.tile_pool(name="opool", bufs=2) as opool,
        ):
            for qt in range(2):
                pa = papool.tile([65, QT], f32, tag="pa")
                pb = pbpool.tile([64, QT], f32, tag="pb")
                for j in range(KC):
                    ps = spool.tile([128, QT], f32, tag="ps")
                    for qq in range(2):
                        nc.tensor.matmul(
                            ps[:, qq * 512 : (qq + 1) * 512],
                            K_t[:, j * 128 : (j + 1) * 128],
                            Q_t[:, qt * QT + qq * 512 : qt * QT + (qq + 1) * 512],
                            start=True,
                            stop=True,
                        )
                    es = epool.tile([128, QT], f32r, tag="es")
                    nc.scalar.activation(es[:], ps[:], Exp)
                    first, last = j == 0, j == KC - 1
                    for qq in range(2):
                        sl = slice(qq * 512, (qq + 1) * 512)
                        nc.tensor.matmul(
                            pb[:, sl],
                            VT[:, j * BW : j * BW + 64],
                            es[:, sl],
                            start=first,
                            stop=last,
                        )
                        nc.tensor.matmul(
                            pa[:, sl],
                            VT[:, j * BW + 64 : j * BW + 129],
                            es[:, sl],
                            start=first,
                            stop=last,
                        )
                sa = opool.tile([65, QT], f32, tag="sa")
                sb = opool.tile([64, QT], f32, tag="sb")
                nc.vector.tensor_copy(sa[:], pa[:])
                nc.vector.tensor_copy(sb[:], pb[:])
                nc.sync.dma_start(oa_d[qt, :, :], sa[:])
                nc.sync.dma_start(ob_d[qt, :, :], sb[:])

    nc.compile()
    return nc


def make_in_maps(x, Wq, Wk, Wv):
    x = np.ascontiguousarray(np.asarray(x, dtype=np.float32).reshape(B, C, N))
    wt = {
        "wq": np.ascontiguousarray(np.asarray(Wq, dtype=np.float32).T),
        "wk": np.ascontiguousarray(np.asarray(Wk, dtype=np.float32).T),
        "wv": np.ascontiguousarray(np.asarray(Wv, dtype=np.float32).T),
    }
    ident = np.eye(128, dtype=np.float32)
    ones = np.ones((128, KC), dtype=np.float32)

    in_maps = []
    for core in range(8):
        b, h = core // 2, core % 2
        in_maps.append(
            {
                "xk": x[b],
                "xq": np.ascontiguousarray(x[b][:, h * MQ : (h + 1) * MQ]),
                "ident": ident,
                "ones": ones,
                **wt,
            }
        )
    return in_maps


def assemble_output(results):
    out = np.empty((B, VC, N), dtype=np.float32)
    for core, r in enumerate(results):
        b, h = core // 2, core % 2
        oa, ob = r["oa"], r["ob"]          # [2, 65, QT], [2, 64, QT]
        un = np.concatenate([oa[:, :64, :], ob], axis=1)   # [2, 128, QT]
        sums = oa[:, 64:65, :]                             # [2, 1, QT]
        core_out = un / sums                               # [2, 128, QT]
        out[b, :, h * MQ : (h + 1) * MQ] = np.concatenate(
            [core_out[0], core_out[1]], axis=1
        )
    return out.reshape(B, VC, H, W)


def kernel(x, Wq, Wk, Wv):
    global _cached_nc
    from concourse.bass_utils import run_bass_kernel_spmd

    if _cached_nc is None:
        _cached_nc = _build()
    in_maps = make_in_maps(x, Wq, Wk, Wv)
    res = run_bass_kernel_spmd(_cached_nc, in_maps, core_ids=list(range(8)))
    return assemble_output(res.results)


# revision 10
# speedup vs baseline: 1.1836x; 1.1836x over previous
"""Trainium2 Bass kernel for nn_AttentionHead.

Computation (per batch b):
    Q = Wq @ x_b, K = Wk @ x_b, V = Wv @ x_b        (x_b: [C=256, N=4096])
    S = Q^T K   [N, N];  A = softmax_k(S)
    out_b = V @ A^T                                  ([VC=128, N])

Sharding: 8 cores = 4 batches x 2 query-halves. Each core computes K/V for
its full batch and Q for its 2048-query half. Inside, a flash-style loop
over 32 key chunks of 128 never materializes the full [4096, 4096] affinity.
The softmax denominators come for free from a ones-column appended to the
PV stationary operand; normalization happens on the host during unshard.
"""

import numpy as np

B, C, VC, H, W = 4, 256, 128, 64, 64
N = H * W            # keys per batch
MQ = N // 2          # queries per core
QT = 1024            # query tile (PSUM-sized)
KC = N // 128        # key chunks of 128
BW = 132             # V^T block width: [64 | 64 | 1 ones | 3 pad] (16B-aligned slices)

_cached_nc = None


def _build():
    from contextlib import ExitStack

    import concourse.bacc as bacc
    import concourse.mybir as mybir
    import concourse.tile as tile

    f32 = mybir.dt.float32
    f32r = mybir.dt.float32r
    Exp = mybir.ActivationFunctionType.Exp

    nc = bacc.Bacc("TRN2", target_bir_lowering=False, debug=False, num_devices=8)

    xk_d = nc.dram_tensor("xk", [C, N], f32r, kind="ExternalInput")
    xq_d = nc.dram_tensor("xq", [C, MQ], f32r, kind="ExternalInput")
    w_d = {
        "wq": nc.dram_tensor("wq", [C, VC], f32r, kind="ExternalInput"),
        "wk": nc.dram_tensor("wk", [C, VC], f32r, kind="ExternalInput"),
        "wv": nc.dram_tensor("wv", [C, VC], f32r, kind="ExternalInput"),
    }
    id_d = nc.dram_tensor("ident", [128, 128], f32, kind="ExternalInput")
    ones_d = nc.dram_tensor("ones", [128, KC], f32r, kind="ExternalInput")
    oa_d = nc.dram_tensor("oa", [2, 65, QT], f32, kind="ExternalOutput")
    ob_d = nc.dram_tensor("ob", [2, 64, QT], f32, kind="ExternalOutput")

    with tile.TileContext(nc) as tc, ExitStack() as ctx:
        persist = ctx.enter_context(tc.tile_pool(name="persist", bufs=1))
        wpool = ctx.enter_context(tc.tile_pool(name="w", bufs=1))

        wts = {}
        for nm in ("wq", "wk", "wv"):
            for cc in range(2):
                t = wpool.tile([128, VC], f32r, tag=f"{nm}{cc}")
                nc.sync.dma_start(t[:], w_d[nm][cc * 128 : (cc + 1) * 128, :])
                wts[(nm, cc)] = t
        ident = wpool.tile([128, 128], f32, tag="ident")
        nc.sync.dma_start(ident[:], id_d[:, :])

        K_t = persist.tile([128, N], f32r, tag="K")
        Q_t = persist.tile([128, MQ], f32r, tag="Q")
        VT = persist.tile([128, KC * BW], f32r, tag="VT")

        with (
            tc.tile_pool(name="xp", bufs=1) as xp,
            tc.tile_pool(name="vp", bufs=1) as vp,
            tc.tile_pool(name="projps", bufs=4, space="PSUM") as pps,
            tc.tile_pool(name="trps", bufs=4, space="PSUM") as tps,
        ):
            xk_t = []
            xq_t = []
            for cc in range(2):
                xkt = xp.tile([128, N], f32r, tag=f"xk{cc}")
                for h in range(8):
                    nc.sync.dma_start(
                        xkt[:, h * 512 : (h + 1) * 512],
                        xk_d[cc * 128 : (cc + 1) * 128, h * 512 : (h + 1) * 512],
                    )
                xk_t.append(xkt)
                xqt = xp.tile([128, MQ], f32r, tag=f"xq{cc}")
                for h in range(4):
                    nc.sync.dma_start(
                        xqt[:, h * 512 : (h + 1) * 512],
                        xq_d[cc * 128 : (cc + 1) * 128, h * 512 : (h + 1) * 512],
                    )
                xq_t.append(xqt)
            V_t = vp.tile([128, N], f32, tag="V")

            def proj(dst, wnm, xt, ncols):
                for t in range(ncols // 512):
                    ps = pps.tile([128, 512], f32, tag="projps")
                    for cc in range(2):
                        nc.tensor.matmul(
                            ps[:],
                            wts[(wnm, cc)][:],
                            xt[cc][:, t * 512 : (t + 1) * 512],
                            start=(cc == 0),
                            stop=(cc == 1),
                        )
                    nc.vector.tensor_copy(dst[:, t * 512 : (t + 1) * 512], ps[:])

            proj(K_t, "wk", xk_t, N)
            proj(Q_t, "wq", xq_t, MQ)
            proj(V_t, "wv", xk_t, N)

            # ones columns (slot 128 of each BW-block) for the softmax sums
            ones_view = VT[:].rearrange("p (j c) -> p j c", c=BW)[:, :, 128:129]
            nc.sync.dma_start(ones_view, ones_d[:, :])

            # V^T blocks: slots [0:64] = V^T[:, 64:128], slots [64:128] = V^T[:, 0:64]
            for j in range(KC):
                tp = tps.tile([128, 128], f32, tag="trps")
                nc.tensor.transpose(tp[:], V_t[:, j * 128 : (j + 1) * 128], ident[:])
                nc.vector.tensor_copy(VT[:, j * BW + 64 : j * BW + 128], tp[:, 0:64])
                nc.vector.tensor_copy(VT[:, j * BW : j * BW + 64], tp[:, 64:128])

        with (
            tc.tile_pool(name="spool", bufs=2, space="PSUM") as spool,
            tc.tile_pool(name="papool", bufs=1, space="PSUM") as papool,
            tc.tile_pool(name="pbpool", bufs=1, space="PSUM") as pbpool,
            tc.tile_pool(name="epool", bufs=6) as epool,
            tc.tile_pool(name="opool", bufs=2) as opool,
        ):
            pairs = [(qt, j) for qt in range(2) for j in range(KC)]
            ps_tiles = {}

            def emit_qk(qt, j):
                ps = spool.tile([128, QT], f32, tag="ps")
                for qq in range(2):
                    nc.tensor.matmul(
                        ps[:, qq * 512 : (qq + 1) * 512],
                        K_t[:, j * 128 : (j + 1) * 128],
                        Q_t[:, qt * QT + qq * 512 : qt * QT + (qq + 1) * 512],
                        start=True,
                        stop=True,
                    )
                ps_tiles[(qt, j)] = ps

            pa = pb = None
            emit_qk(*pairs[0])
            for i, (qt, j) in enumerate(pairs):
                if i + 1 < len(pairs):
                    emit_qk(*pairs[i + 1])
                if j == 0:
                    pa = papool.tile([65, QT], f32, tag="pa")
                    pb = pbpool.tile([64, QT], f32, tag="pb")
                ps = ps_tiles.pop((qt, j))
                es = epool.tile([128, QT], f32r, tag="es")
                nc.scalar.activation(es[:], ps[:], Exp)
                first, last = j == 0, j == KC - 1
                for qq in range(2):
                    sl = slice(qq * 512, (qq + 1) * 512)
                    nc.tensor.matmul(
                        pb[:, sl],
                        VT[:, j * BW : j * BW + 64],
                        es[:, sl],
                        start=first,
                        stop=last,
                    )
                    nc.tensor.matmul(
                        pa[:, sl],
                        VT[:, j * BW + 64 : j * BW + 129],
                        es[:, sl],
                        start=first,
                        stop=last,
                    )
                if last:
                    sa = opool.tile([65, QT], f32, tag="sa")
                    sb = opool.tile([64, QT], f32, tag="sb")
                    nc.vector.tensor_copy(sa[:], pa[:])
                    nc.vector.tensor_copy(sb[:], pb[:])
                    nc.sync.dma_start(oa_d[qt, :, :], sa[:])
                    nc.sync.dma_start(ob_d[qt, :, :], sb[:])

    nc.compile()
    return nc


def make_in_maps(x, Wq, Wk, Wv):
    x = np.ascontiguousarray(np.asarray(x, dtype=np.float32).reshape(B, C, N))
    wt = {
        "wq": np.ascontiguousarray(np.asarray(Wq, dtype=np.float32).T),
        "wk": np.ascontiguousarray(np.asarray(Wk, dtype=np.float32).T),
        "wv": np.ascontiguousarray(np.asarray(Wv, dtype=np.float32).T),
    }
    ident = np.eye(128, dtype=np.float32)
    ones = np.ones((128, KC), dtype=np.float32)

    in_maps = []
    for core in range(8):
        b, h = core // 2, core % 2
        in_maps.append(
            {
                "xk": x[b],
                "xq": np.ascontiguousarray(x[b][:, h * MQ : (h + 1) * MQ]),
                "ident": ident,
                "ones": ones,
                **wt,
            }
        )
    return in_maps


def assemble_output(results):
    out = np.empty((B, VC, N), dtype=np.float32)
    for core, r in enumerate(results):
        b, h = core // 2, core % 2
        oa, ob = r["oa"], r["ob"]          # [2, 65, QT], [2, 64, QT]
        un = np.concatenate([oa[:, :64, :], ob], axis=1)   # [2, 128, QT]
        sums = oa[:, 64:65, :]                             # [2, 1, QT]
        core_out = un / sums                               # [2, 128, QT]
        out[b, :, h * MQ : (h + 1) * MQ] = np.concatenate(
            [core_out[0], core_out[1]], axis=1
        )
    return out.reshape(B, VC, H, W)


def kernel(x, Wq, Wk, Wv):
    global _cached_nc
    from concourse.bass_utils import run_bass_kernel_spmd

    if _cached_nc is None:
        _cached_nc = _build()
    in_maps = make_in_maps(x, Wq, Wk, Wv)
    res = run_bass_kernel_spmd(_cached_nc, in_maps, core_ids=list(range(8)))
    return assemble_output(res.results)


# revision 12
# speedup vs baseline: 1.2415x; 1.0489x over previous
"""Trainium2 Bass kernel for nn_AttentionHead.

Computation (per batch b):
    Q = Wq @ x_b, K = Wk @ x_b, V = Wv @ x_b        (x_b: [C=256, N=4096])
    S = Q^T K   [N, N];  A = softmax_k(S)
    out_b = V @ A^T                                  ([VC=128, N])

Sharding: 8 cores = 4 batches x 2 query-halves. Each core computes K/V for
its full batch and Q for its 2048-query half. Inside, a flash-style loop
over 32 key chunks of 128 never materializes the full [4096, 4096] affinity.
The softmax denominators come for free from a ones-column appended to the
PV stationary operand; normalization happens on the host during unshard.
"""

import numpy as np

B, C, VC, H, W = 4, 256, 128, 64, 64
N = H * W            # keys per batch
MQ = N // 2          # queries per core
QT = 1024            # query tile (PSUM-sized)
KC = N // 128        # key chunks of 128
BW = 132             # V^T block width: [64 | 64 | 1 ones | 3 pad] (16B-aligned slices)

_cached_nc = None


def _build():
    from contextlib import ExitStack

    import concourse.bacc as bacc
    import concourse.mybir as mybir
    import concourse.tile as tile

    f32 = mybir.dt.float32
    f32r = mybir.dt.float32r
    Exp = mybir.ActivationFunctionType.Exp

    nc = bacc.Bacc("TRN2", target_bir_lowering=False, debug=False, num_devices=8)

    xk_d = nc.dram_tensor("xk", [C, N], f32r, kind="ExternalInput")
    xq_d = nc.dram_tensor("xq", [C, MQ], f32r, kind="ExternalInput")
    w_d = {
        "wq": nc.dram_tensor("wq", [C, VC], f32r, kind="ExternalInput"),
        "wk": nc.dram_tensor("wk", [C, VC], f32r, kind="ExternalInput"),
        "wv": nc.dram_tensor("wv", [C, VC], f32r, kind="ExternalInput"),
    }
    ones_d = nc.dram_tensor("ones", [128, KC], f32r, kind="ExternalInput")
    oa_d = nc.dram_tensor("oa", [2, 65, QT], f32, kind="ExternalOutput")
    ob_d = nc.dram_tensor("ob", [2, 64, QT], f32, kind="ExternalOutput")

    with tile.TileContext(nc) as tc, ExitStack() as ctx:
        persist = ctx.enter_context(tc.tile_pool(name="persist", bufs=1))
        wpool = ctx.enter_context(tc.tile_pool(name="w", bufs=1))

        wts = {}
        for nm in ("wq", "wk", "wv"):
            for cc in range(2):
                t = wpool.tile([128, VC], f32r, tag=f"{nm}{cc}")
                nc.gpsimd.dma_start(t[:], w_d[nm][cc * 128 : (cc + 1) * 128, :])
                wts[(nm, cc)] = t
        K_t = persist.tile([128, N], f32r, tag="K")
        Q_t = persist.tile([128, MQ], f32r, tag="Q")
        VT = persist.tile([128, KC * BW], f32r, tag="VT")

        with (
            tc.tile_pool(name="xp", bufs=1) as xp,
            tc.tile_pool(name="projps", bufs=4, space="PSUM") as pps,
            tc.tile_pool(name="trps", bufs=4, space="PSUM") as tps,
        ):
            xk_t = [
                xp.tile([128, N], f32r, tag=f"xk{cc}", name=f"xk{cc}")
                for cc in range(2)
            ]
            xq_t = [
                xp.tile([128, MQ], f32r, tag=f"xq{cc}", name=f"xq{cc}")
                for cc in range(2)
            ]
            for h in range(4):
                for cc in range(2):
                    nc.sync.dma_start(
                        xk_t[cc][:, h * 1024 : (h + 1) * 1024],
                        xk_d[cc * 128 : (cc + 1) * 128, h * 1024 : (h + 1) * 1024],
                    )
            for h in range(2):
                for cc in range(2):
                    nc.gpsimd.dma_start(
                        xq_t[cc][:, h * 1024 : (h + 1) * 1024],
                        xq_d[cc * 128 : (cc + 1) * 128, h * 1024 : (h + 1) * 1024],
                    )
            def proj(dst, wnm, xt, ncols):
                for t in range(ncols // 512):
                    ps = pps.tile([128, 512], f32, tag="projps")
                    for cc in range(2):
                        nc.tensor.matmul(
                            ps[:],
                            wts[(wnm, cc)][:],
                            xt[cc][:, t * 512 : (t + 1) * 512],
                            start=(cc == 0),
                            stop=(cc == 1),
                        )
                    nc.vector.tensor_copy(dst[:, t * 512 : (t + 1) * 512], ps[:])

            proj(K_t, "wk", xk_t, N)
            proj(Q_t, "wq", xq_t, MQ)

            # ones columns (slot 128 of each BW-block) for the softmax sums
            ones_view = VT[:].rearrange("p (j c) -> p j c", c=BW)[:, :, 128:129]
            nc.gpsimd.dma_start(ones_view, ones_d[:, :])

            # V^T blocks via direct matmul: out [n-block, d] = x_block.T @ Wv.T
            # slots [0:64] = V^T[:, 64:128], slots [64:128] = V^T[:, 0:64]
            for j in range(KC):
                tp = tps.tile([128, 128], f32, tag="trps")
                for cc in range(2):
                    nc.tensor.matmul(
                        tp[:],
                        xk_t[cc][:, j * 128 : (j + 1) * 128],
                        wts[("wv", cc)][:],
                        start=(cc == 0),
                        stop=(cc == 1),
                    )
                nc.vector.tensor_copy(VT[:, j * BW + 64 : j * BW + 128], tp[:, 0:64])
                nc.vector.tensor_copy(VT[:, j * BW : j * BW + 64], tp[:, 64:128])

        with (
            tc.tile_pool(name="spool", bufs=2, space="PSUM") as spool,
            tc.tile_pool(name="papool", bufs=1, space="PSUM") as papool,
            tc.tile_pool(name="pbpool", bufs=1, space="PSUM") as pbpool,
            tc.tile_pool(name="epool", bufs=6) as epool,
            tc.tile_pool(name="opool", bufs=2) as opool,
        ):
            pairs = [(qt, j) for qt in range(2) for j in range(KC)]
            ps_tiles = {}

            def emit_qk(qt, j):
                ps = spool.tile([128, QT], f32, tag="ps")
                for qq in range(2):
                    nc.tensor.matmul(
                        ps[:, qq * 512 : (qq + 1) * 512],
                        K_t[:, j * 128 : (j + 1) * 128],
                        Q_t[:, qt * QT + qq * 512 : qt * QT + (qq + 1) * 512],
                        start=True,
                        stop=True,
                    )
                ps_tiles[(qt, j)] = ps

            pa = pb = None
            emit_qk(*pairs[0])
            for i, (qt, j) in enumerate(pairs):
                if i + 1 < len(pairs):
                    emit_qk(*pairs[i + 1])
                if j == 0:
                    pa = papool.tile([65, QT], f32, tag="pa")
                    pb = pbpool.tile([64, QT], f32, tag="pb")
                ps = ps_tiles.pop((qt, j))
                es = epool.tile([128, QT], f32r, tag="es")
                nc.scalar.activation(es[:], ps[:], Exp)
                first, last = j == 0, j == KC - 1
                for qq in range(2):
                    sl = slice(qq * 512, (qq + 1) * 512)
                    nc.tensor.matmul(
                        pb[:, sl],
                        VT[:, j * BW : j * BW + 64],
                        es[:, sl],
                        start=first,
                        stop=last,
                    )
                    nc.tensor.matmul(
                        pa[:, sl],
                        VT[:, j * BW + 64 : j * BW + 129],
                        es[:, sl],
                        start=first,
                        stop=last,
                    )
                if last:
                    sa = opool.tile([65, QT], f32, tag="sa")
                    sb = opool.tile([64, QT], f32, tag="sb")
                    nc.vector.tensor_copy(sa[:], pa[:])
                    nc.vector.tensor_copy(sb[:], pb[:])
                    nc.sync.dma_start(oa_d[qt, :, :], sa[:])
                    nc.sync.dma_start(ob_d[qt, :, :], sb[:])

    nc.compile()
    return nc


def make_in_maps(x, Wq, Wk, Wv):
    x = np.ascontiguousarray(np.asarray(x, dtype=np.float32).reshape(B, C, N))
    wt = {
        "wq": np.ascontiguousarray(np.asarray(Wq, dtype=np.float32).T),
        "wk": np.ascontiguousarray(np.asarray(Wk, dtype=np.float32).T),
        "wv": np.ascontiguousarray(np.asarray(Wv, dtype=np.float32).T),
    }
    ones = np.ones((128, KC), dtype=np.float32)

    in_maps = []
    for core in range(8):
        b, h = core // 2, core % 2
        in_maps.append(
            {
                "xk": x[b],
                "xq": np.ascontiguousarray(x[b][:, h * MQ : (h + 1) * MQ]),
                "ones": ones,
                **wt,
            }
        )
    return in_maps


def assemble_output(results):
    out = np.empty((B, VC, N), dtype=np.float32)
    for core, r in enumerate(results):
        b, h = core // 2, core % 2
        oa, ob = r["oa"], r["ob"]          # [2, 65, QT], [2, 64, QT]
        un = np.concatenate([oa[:, :64, :], ob], axis=1)   # [2, 128, QT]
        sums = oa[:, 64:65, :]                             # [2, 1, QT]
        core_out = un / sums                               # [2, 128, QT]
        out[b, :, h * MQ : (h + 1) * MQ] = np.concatenate(
            [core_out[0], core_out[1]], axis=1
        )
    return out.reshape(B, VC, H, W)


def kernel(x, Wq, Wk, Wv):
    global _cached_nc
    from concourse.bass_utils import run_bass_kernel_spmd

    if _cached_nc is None:
        _cached_nc = _build()
    in_maps = make_in_maps(x, Wq, Wk, Wv)
    res = run_bass_kernel_spmd(_cached_nc, in_maps, core_ids=list(range(8)))
    return assemble_output(res.results)


# revision 13
# speedup vs baseline: 1.3183x; 1.0619x over previous
"""Trainium2 Bass kernel for nn_AttentionHead.

Computation (per batch b):
    Q = Wq @ x_b, K = Wk @ x_b, V = Wv @ x_b        (x_b: [C=256, N=4096])
    S = Q^T K   [N, N];  A = softmax_k(S)
    out_b = V @ A^T                                  ([VC=128, N])

Sharding: 8 cores = 4 batches x 2 query-halves. Each core computes K/V^T for
its full batch and Q for its 2048-query half; a flash-style loop over 32 key
chunks of 128 never materializes the full [4096, 4096] affinity. All matmuls
run in fp32r (full PE rate at 512-wide moving operands). Softmax denominators:
exp tiles are pair/quad-summed on VectorE, then a tiny M=1 ones-matmul
contracts the 8 quad partials in PSUM; normalization happens on the host.
"""

import numpy as np

B, C, VC, H, W = 4, 256, 128, 64, 64
N = H * W            # keys per batch
MQ = N // 2          # queries per core
QT = 1024            # query tile (PSUM-sized)
KC = N // 128        # key chunks of 128

_cached_nc = None


def _build():
    from contextlib import ExitStack

    import concourse.bacc as bacc
    import concourse.mybir as mybir
    import concourse.tile as tile

    f32 = mybir.dt.float32
    f32r = mybir.dt.float32r
    Exp = mybir.ActivationFunctionType.Exp

    nc = bacc.Bacc("TRN2", target_bir_lowering=False, debug=False, num_devices=8)

    xk_d = nc.dram_tensor("xk", [C, N], f32r, kind="ExternalInput")
    xq_d = nc.dram_tensor("xq", [C, MQ], f32r, kind="ExternalInput")
    w_d = {
        "wq": nc.dram_tensor("wq", [C, VC], f32r, kind="ExternalInput"),
        "wk": nc.dram_tensor("wk", [C, VC], f32r, kind="ExternalInput"),
        "wv": nc.dram_tensor("wv", [C, VC], f32r, kind="ExternalInput"),
    }
    ones_d = nc.dram_tensor("ones", [128, 1], f32r, kind="ExternalInput")
    oc_d = nc.dram_tensor("oc", [2, 128, QT], f32, kind="ExternalOutput")
    os_d = nc.dram_tensor("osum", [2, 1, QT], f32, kind="ExternalOutput")

    with tile.TileContext(nc) as tc, ExitStack() as ctx:
        persist = ctx.enter_context(tc.tile_pool(name="persist", bufs=1))
        wpool = ctx.enter_context(tc.tile_pool(name="w", bufs=1))

        wts = {}
        for nm in ("wq", "wk", "wv"):
            for cc in range(2):
                t = wpool.tile([128, VC], f32r, tag=f"{nm}{cc}")
                nc.gpsimd.dma_start(t[:], w_d[nm][cc * 128 : (cc + 1) * 128, :])
                wts[(nm, cc)] = t
        ones_t = wpool.tile([128, 1], f32r, tag="ones")
        nc.gpsimd.dma_start(ones_t[:], ones_d[:, :])

        K_t = persist.tile([128, N], f32r, tag="K")
        Q_t = persist.tile([128, MQ], f32r, tag="Q")
        VT = persist.tile([128, KC * 128], f32r, tag="VT")

        with (
            tc.tile_pool(name="xp", bufs=1) as xp,
            tc.tile_pool(name="projps", bufs=4, space="PSUM") as pps,
            tc.tile_pool(name="trps", bufs=4, space="PSUM") as tps,
        ):
            xk_t = [
                xp.tile([128, N], f32r, tag=f"xk{cc}", name=f"xk{cc}")
                for cc in range(2)
            ]
            xq_t = [
                xp.tile([128, MQ], f32r, tag=f"xq{cc}", name=f"xq{cc}")
                for cc in range(2)
            ]
            # xq first (gates Q proj, which gates the first attention chunk),
            # then xk pieces in consumption order; weights go via gpsimd.
            for h in range(2):
                for cc in range(2):
                    nc.sync.dma_start(
                        xq_t[cc][:, h * 1024 : (h + 1) * 1024],
                        xq_d[cc * 128 : (cc + 1) * 128, h * 1024 : (h + 1) * 1024],
                    )
            for h in range(4):
                for cc in range(2):
                    nc.sync.dma_start(
                        xk_t[cc][:, h * 1024 : (h + 1) * 1024],
                        xk_d[cc * 128 : (cc + 1) * 128, h * 1024 : (h + 1) * 1024],
                    )

            def proj(dst, wnm, xt, ncols):
                for t in range(ncols // 512):
                    ps = pps.tile([128, 512], f32, tag="projps")
                    for cc in range(2):
                        nc.tensor.matmul(
                            ps[:],
                            wts[(wnm, cc)][:],
                            xt[cc][:, t * 512 : (t + 1) * 512],
                            start=(cc == 0),
                            stop=(cc == 1),
                        )
                    nc.vector.tensor_copy(dst[:, t * 512 : (t + 1) * 512], ps[:])

            proj(Q_t, "wq", xq_t, MQ)
            proj(K_t, "wk", xk_t, N)

            # V^T blocks via direct matmul: out [n-block, d] = x_block.T @ Wv.T
            for j in range(KC):
                tp = tps.tile([128, 128], f32, tag="trps")
                for cc in range(2):
                    nc.tensor.matmul(
                        tp[:],
                        xk_t[cc][:, j * 128 : (j + 1) * 128],
                        wts[("wv", cc)][:],
                        start=(cc == 0),
                        stop=(cc == 1),
                    )
                nc.vector.tensor_copy(VT[:, j * 128 : (j + 1) * 128], tp[:])

        with (
            tc.tile_pool(name="spool", bufs=2, space="PSUM") as spool,
            tc.tile_pool(name="pcpool", bufs=1, space="PSUM") as pcpool,
            tc.tile_pool(name="smpool", bufs=1, space="PSUM") as smpool,
            tc.tile_pool(name="epool", bufs=6) as epool,
            tc.tile_pool(name="prpool", bufs=2) as prpool,
            tc.tile_pool(name="qdpool", bufs=2) as qdpool,
            tc.tile_pool(name="opool", bufs=2) as opool,
        ):
            pairs = [(qt, j) for qt in range(2) for j in range(KC)]
            ps_tiles = {}

            def emit_qk(qt, j):
                ps = spool.tile([128, QT], f32, tag="ps")
                for qq in range(2):
                    nc.tensor.matmul(
                        ps[:, qq * 512 : (qq + 1) * 512],
                        K_t[:, j * 128 : (j + 1) * 128],
                        Q_t[:, qt * QT + qq * 512 : qt * QT + (qq + 1) * 512],
                        start=True,
                        stop=True,
                    )
                ps_tiles[(qt, j)] = ps

            pc = sm = None
            es_prev = pr_prev = None
            emit_qk(*pairs[0])
            for i, (qt, j) in enumerate(pairs):
                if i + 1 < len(pairs):
                    emit_qk(*pairs[i + 1])
                if j == 0:
                    pc = pcpool.tile([128, QT], f32, tag="pc")
                    sm = smpool.tile([1, QT], f32, tag="sm")
                ps = ps_tiles.pop((qt, j))
                es = epool.tile([128, QT], f32r, tag="es")
                nc.scalar.activation(es[:], ps[:], Exp)
                first, last = j == 0, j == KC - 1
                for qq in range(2):
                    sl = slice(qq * 512, (qq + 1) * 512)
                    nc.tensor.matmul(
                        pc[:, sl],
                        VT[:, j * 128 : (j + 1) * 128],
                        es[:, sl],
                        start=first,
                        stop=last,
                    )
                # softmax denominators: pair/quad partial sums on DVE,
                # then an M=1 ones-matmul contracts each quad into PSUM
                if j % 2 == 0:
                    es_prev = es
                else:
                    pr = prpool.tile([128, QT], f32r, tag="pr")
                    nc.vector.tensor_add(pr[:], es_prev[:], es[:])
                    if j % 4 == 1:
                        pr_prev = pr
                    else:
                        qd = qdpool.tile([128, QT], f32r, tag="qd")
                        nc.vector.tensor_add(qd[:], pr_prev[:], pr[:])
                        for qq in range(2):
                            sl = slice(qq * 512, (qq + 1) * 512)
                            nc.tensor.matmul(
                                sm[:, sl],
                                ones_t[:],
                                qd[:, sl],
                                start=(j == 3),
                                stop=(j == KC - 1),
                            )
                if last:
                    so = opool.tile([128, QT], f32, tag="so")
                    ss = opool.tile([1, QT], f32, tag="ss")
                    nc.vector.tensor_copy(so[:], pc[:])
                    nc.scalar.copy(ss[:], sm[:])
                    nc.sync.dma_start(oc_d[qt, :, :], so[:])
                    nc.sync.dma_start(os_d[qt, :, :], ss[:])

    nc.compile()
    return nc


def make_in_maps(x, Wq, Wk, Wv):
    x = np.ascontiguousarray(np.asarray(x, dtype=np.float32).reshape(B, C, N))
    wt = {
        "wq": np.ascontiguousarray(np.asarray(Wq, dtype=np.float32).T),
        "wk": np.ascontiguousarray(np.asarray(Wk, dtype=np.float32).T),
        "wv": np.ascontiguousarray(np.asarray(Wv, dtype=np.float32).T),
    }
    ones = np.ones((128, 1), dtype=np.float32)

    in_maps = []
    for core in range(8):
        b, h = core // 2, core % 2
        in_maps.append(
            {
                "xk": x[b],
                "xq": np.ascontiguousarray(x[b][:, h * MQ : (h + 1) * MQ]),
                "ones": ones,
                **wt,
            }
        )
    return in_maps


def assemble_output(results):
    out = np.empty((B, VC, N), dtype=np.float32)
    for core, r in enumerate(results):
        b, h = core // 2, core % 2
        core_out = r["oc"] / r["osum"]                     # [2, 128, QT]
        out[b, :, h * MQ : (h + 1) * MQ] = np.concatenate(
            [core_out[0], core_out[1]], axis=1
        )
    return out.reshape(B, VC, H, W)


def kernel(x, Wq, Wk, Wv):
    global _cached_nc
    from concourse.bass_utils import run_bass_kernel_spmd

    if _cached_nc is None:
        _cached_nc = _build()
    in_maps = make_in_maps(x, Wq, Wk, Wv)
    res = run_bass_kernel_spmd(_cached_nc, in_maps, core_ids=list(range(8)))
    return assemble_output(res.results)


# revision 15
# speedup vs baseline: 1.4354x; 1.0889x over previous
"""Trainium2 Bass kernel for nn_AttentionHead.

Computation (per batch b):
    Q = Wq @ x_b, K = Wk @ x_b, V = Wv @ x_b        (x_b: [C=256, N=4096])
    S = Q^T K   [N, N];  A = softmax_k(S)
    out_b = V @ A^T                                  ([VC=128, N])

Sharding: 8 cores = 4 batches x 2 query-halves. Each core computes K/V^T for
its full batch and Q for its 2048-query half; a flash-style loop over 32 key
chunks of 128 never materializes the full [4096, 4096] affinity.

Numerics: QK logits in fp32r (full PE rate, near-fp32 accuracy pre-exp);
exp tiles and V^T in bf16 (linear path, errors stay ~0.3%). Softmax
denominators: exp tiles are tree-summed pairwise on VectorE down to one
[128, QT] partial per query-half; the final 128-way reduction and the
normalization happen on the host during unshard.
"""

import numpy as np

B, C, VC, H, W = 4, 256, 128, 64, 64
N = H * W            # keys per batch
MQ = N // 2          # queries per core
QT = 1024            # query tile (PSUM-sized)
KC = N // 128        # key chunks of 128
VT_UPFRONT = 20      # V^T blocks built before the attention loop

_cached_nc = None


def _build():
    from contextlib import ExitStack

    import concourse.bacc as bacc
    import concourse.mybir as mybir
    import concourse.tile as tile

    f32 = mybir.dt.float32
    f32r = mybir.dt.float32r
    bf16 = mybir.dt.bfloat16
    Exp = mybir.ActivationFunctionType.Exp

    nc = bacc.Bacc("TRN2", target_bir_lowering=False, debug=False, num_devices=8)

    xk_d = nc.dram_tensor("xk", [C, N], f32r, kind="ExternalInput")
    xq_d = nc.dram_tensor("xq", [C, MQ], f32r, kind="ExternalInput")
    w_d = {
        "wq": nc.dram_tensor("wq", [C, VC], f32r, kind="ExternalInput"),
        "wk": nc.dram_tensor("wk", [C, VC], f32r, kind="ExternalInput"),
        "wv": nc.dram_tensor("wv", [C, VC], f32r, kind="ExternalInput"),
    }
    oc_d = nc.dram_tensor("oc", [2, 128, QT], f32, kind="ExternalOutput")
    oss_d = nc.dram_tensor("oss", [2, 128, QT], bf16, kind="ExternalOutput")

    with tile.TileContext(nc) as tc, ExitStack() as ctx:
        persist = ctx.enter_context(tc.tile_pool(name="persist", bufs=1))
        wpool = ctx.enter_context(tc.tile_pool(name="w", bufs=1))
        xp = ctx.enter_context(tc.tile_pool(name="xp", bufs=1))
        tps = ctx.enter_context(tc.tile_pool(name="trps", bufs=2, space="PSUM"))

        wts = {}
        for nm in ("wq", "wk", "wv"):
            for cc in range(2):
                t = wpool.tile([128, VC], f32r, tag=f"{nm}{cc}")
                nc.gpsimd.dma_start(t[:], w_d[nm][cc * 128 : (cc + 1) * 128, :])
                wts[(nm, cc)] = t

        K_t = persist.tile([128, N], f32r, tag="K")
        Q_t = persist.tile([128, MQ], f32r, tag="Q")
        VT = persist.tile([128, KC * 128], bf16, tag="VT")

        xk_t = [
            xp.tile([128, N], f32r, tag=f"xk{cc}", name=f"xk{cc}") for cc in range(2)
        ]
        xq_t = [
            xp.tile([128, MQ], f32r, tag=f"xq{cc}", name=f"xq{cc}") for cc in range(2)
        ]
        # xq first (gates Q proj -> first attention chunk), then xk pieces in
        # consumption order; weights via gpsimd.
        for h in range(2):
            for cc in range(2):
                nc.sync.dma_start(
                    xq_t[cc][:, h * 1024 : (h + 1) * 1024],
                    xq_d[cc * 128 : (cc + 1) * 128, h * 1024 : (h + 1) * 1024],
                )
        for h in range(4):
            for cc in range(2):
                nc.sync.dma_start(
                    xk_t[cc][:, h * 1024 : (h + 1) * 1024],
                    xk_d[cc * 128 : (cc + 1) * 128, h * 1024 : (h + 1) * 1024],
                )

        def emit_vt(j):
            # V^T block j: [n-block, d] = x_block.T @ Wv.T
            tp = tps.tile([128, 128], f32, tag="trps", name="tp")
            for cc in range(2):
                nc.tensor.matmul(
                    tp[:],
                    xk_t[cc][:, j * 128 : (j + 1) * 128],
                    wts[("wv", cc)][:],
                    start=(cc == 0),
                    stop=(cc == 1),
                )
            nc.vector.tensor_copy(VT[:, j * 128 : (j + 1) * 128], tp[:])

        with tc.tile_pool(name="projps", bufs=4, space="PSUM") as pps:

            def proj(dst, wnm, xt, ncols):
                for t in range(ncols // 512):
                    ps = pps.tile([128, 512], f32, tag="projps", name="ps")
                    for cc in range(2):
                        nc.tensor.matmul(
                            ps[:],
                            wts[(wnm, cc)][:],
                            xt[cc][:, t * 512 : (t + 1) * 512],
                            start=(cc == 0),
                            stop=(cc == 1),
                        )
                    nc.vector.tensor_copy(dst[:, t * 512 : (t + 1) * 512], ps[:])

            proj(Q_t, "wq", xq_t, MQ)
            proj(K_t, "wk", xk_t, N)
            for j in range(VT_UPFRONT):
                emit_vt(j)

        with (
            tc.tile_pool(name="spool", bufs=2, space="PSUM") as spool,
            tc.tile_pool(name="pcpool", bufs=1, space="PSUM") as pcpool,
            tc.tile_pool(name="epool", bufs=6) as epool,
            tc.tile_pool(name="treep", bufs=2) as treep,
            tc.tile_pool(name="opool", bufs=2) as opool,
        ):
            pairs = [(qt, j) for qt in range(2) for j in range(KC)]
            ps_tiles = {}

            def emit_qk(qt, j):
                ps = spool.tile([128, QT], f32, tag="ps", name="ps")
                for qq in range(2):
                    nc.tensor.matmul(
                        ps[:, qq * 512 : (qq + 1) * 512],
                        K_t[:, j * 128 : (j + 1) * 128],
                        Q_t[:, qt * QT + qq * 512 : qt * QT + (qq + 1) * 512],
                        start=True,
                        stop=True,
                    )
                ps_tiles[(qt, j)] = ps

            # binary-counter pairwise reduction of exp tiles on DVE
            pending = []

            def tree_push(t, level=0):
                while pending and pending[-1][0] == level:
                    _, other = pending.pop()
                    nt = treep.tile(
                        [128, QT], bf16, tag=f"l{level + 1}", name=f"tl{level + 1}"
                    )
                    nc.vector.tensor_add(nt[:], other[:], t[:])
                    t, level = nt, level + 1
                pending.append((level, t))

            # lazy V^T: one matmul per early chunk builds blocks VT_UPFRONT..31
            vt_lazy = [
                (j, cc) for j in range(VT_UPFRONT, KC) for cc in range(2)
            ]

            pc = None
            emit_qk(*pairs[0])
            for i, (qt, j) in enumerate(pairs):
                if i + 1 < len(pairs):
                    emit_qk(*pairs[i + 1])
                if i < len(vt_lazy):
                    jj, cc = vt_lazy[i]
                    if cc == 0:
                        vt_tp = tps.tile([128, 128], f32, tag="trps", name="tp")
                    nc.tensor.matmul(
                        vt_tp[:],
                        xk_t[cc][:, jj * 128 : (jj + 1) * 128],
                        wts[("wv", cc)][:],
                        start=(cc == 0),
                        stop=(cc == 1),
                    )
                    if cc == 1:
                        nc.vector.tensor_copy(
                            VT[:, jj * 128 : (jj + 1) * 128], vt_tp[:]
                        )
                if j == 0:
                    pc = pcpool.tile([128, QT], f32, tag="pc", name="pc")
                ps = ps_tiles.pop((qt, j))
                es = epool.tile([128, QT], bf16, tag="es", name="es")
                nc.scalar.activation(es[:], ps[:], Exp)
                first, last = j == 0, j == KC - 1
                for qq in range(2):
                    sl = slice(qq * 512, (qq + 1) * 512)
                    nc.tensor.matmul(
                        pc[:, sl],
                        VT[:, j * 128 : (j + 1) * 128],
                        es[:, sl],
                        start=first,
                        stop=last,
                    )
                tree_push(es)
                if last:
                    acc = pending.pop()[1]
                    pending.clear()
                    so = opool.tile([128, QT], f32, tag="so", name="so")
                    nc.vector.tensor_copy(so[:], pc[:])
                    nc.sync.dma_start(oc_d[qt, :, :], so[:])
                    nc.sync.dma_start(oss_d[qt, :, :], acc[:])

    nc.compile()
    return nc


def make_in_maps(x, Wq, Wk, Wv):
    x = np.ascontiguousarray(np.asarray(x, dtype=np.float32).reshape(B, C, N))
    wt = {
        "wq": np.ascontiguousarray(np.asarray(Wq, dtype=np.float32).T),
        "wk": np.ascontiguousarray(np.asarray(Wk, dtype=np.float32).T),
        "wv": np.ascontiguousarray(np.asarray(Wv, dtype=np.float32).T),
    }

    in_maps = []
    for core in range(8):
        b, h = core // 2, core % 2
        in_maps.append(
            {
                "xk": x[b],
                "xq": np.ascontiguousarray(x[b][:, h * MQ : (h + 1) * MQ]),
                **wt,
            }
        )
    return in_maps


def assemble_output(results):
    out = np.empty((B, VC, N), dtype=np.float32)
    for core, r in enumerate(results):
        b, h = core // 2, core % 2
        sums = r["oss"].astype(np.float32).sum(axis=1, keepdims=True)  # [2,1,QT]
        core_out = r["oc"] / sums                                     # [2,128,QT]
        out[b, :, h * MQ : (h + 1) * MQ] = np.concatenate(
            [core_out[0], core_out[1]], axis=1
        )
    return out.reshape(B, VC, H, W)


def kernel(x, Wq, Wk, Wv):
    global _cached_nc
    from concourse.bass_utils import run_bass_kernel_spmd

    if _cached_nc is None:
        _cached_nc = _build()
    in_maps = make_in_maps(x, Wq, Wk, Wv)
    res = run_bass_kernel_spmd(_cached_nc, in_maps, core_ids=list(range(8)))
    return assemble_output(res.results)


# revision 16
# speedup vs baseline: 1.5436x; 1.0753x over previous
"""Trainium2 Bass kernel for nn_AttentionHead.

Computation (per batch b):
    Q = Wq @ x_b, K = Wk @ x_b, V = Wv @ x_b        (x_b: [C=256, N=4096])
    S = Q^T K   [N, N];  A = softmax_k(S)
    out_b = V @ A^T                                  ([VC=128, N])

Sharding: 8 cores = 4 batches x 2 query-halves. Each core computes K/V^T for
its full batch and Q for its 2048-query half; a flash-style loop over 32 key
chunks of 128 never materializes the full [4096, 4096] affinity.

Numerics: QK logits in fp32r (full PE rate, near-fp32 accuracy pre-exp);
exp tiles and V^T in bf16 (linear path, errors stay ~0.3%). Softmax
denominators: exp tiles are tree-summed pairwise on VectorE down to one
[128, QT] partial per query-half; the final 128-way reduction and the
normalization happen on the host during unshard.
"""

import numpy as np

B, C, VC, H, W = 4, 256, 128, 64, 64
N = H * W            # keys per batch
MQ = N // 2          # queries per core
QT = 1024            # query tile (PSUM-sized)
KC = N // 128        # key chunks of 128
VT_UPFRONT = 20      # V^T blocks built before the attention loop

_cached_nc = None


def _build():
    from contextlib import ExitStack

    import concourse.bacc as bacc
    import concourse.mybir as mybir
    import concourse.tile as tile

    f32 = mybir.dt.float32
    f32r = mybir.dt.float32r
    bf16 = mybir.dt.bfloat16
    Exp = mybir.ActivationFunctionType.Exp

    nc = bacc.Bacc("TRN2", target_bir_lowering=False, debug=False, num_devices=8)

    xk_d = nc.dram_tensor("xk", [C, N], f32r, kind="ExternalInput")
    xq_d = nc.dram_tensor("xq", [C, MQ], f32r, kind="ExternalInput")
    w_d = {
        "wq": nc.dram_tensor("wq", [C, VC], f32r, kind="ExternalInput"),
        "wk": nc.dram_tensor("wk", [C, VC], f32r, kind="ExternalInput"),
        "wv": nc.dram_tensor("wv", [C, VC], f32r, kind="ExternalInput"),
    }
    oc_d = nc.dram_tensor("oc", [2, 128, QT], f32, kind="ExternalOutput")
    oss_d = nc.dram_tensor("oss", [2, 128, QT], bf16, kind="ExternalOutput")

    with tile.TileContext(nc) as tc, ExitStack() as ctx:
        persist = ctx.enter_context(tc.tile_pool(name="persist", bufs=1))
        wpool = ctx.enter_context(tc.tile_pool(name="w", bufs=1))
        xp = ctx.enter_context(tc.tile_pool(name="xp", bufs=1))

        wts = {}
        for nm in ("wq", "wk", "wv"):
            for cc in range(2):
                t = wpool.tile([128, VC], f32r, tag=f"{nm}{cc}")
                nc.gpsimd.dma_start(t[:], w_d[nm][cc * 128 : (cc + 1) * 128, :])
                wts[(nm, cc)] = t

        K_t = persist.tile([128, N], f32r, tag="K")
        Q_t = persist.tile([128, MQ], f32r, tag="Q")
        VT = persist.tile([128, KC * 128], bf16, tag="VT")

        xk_t = [
            xp.tile([128, N], f32r, tag=f"xk{cc}", name=f"xk{cc}") for cc in range(2)
        ]
        xq_t = [
            xp.tile([128, MQ], f32r, tag=f"xq{cc}", name=f"xq{cc}") for cc in range(2)
        ]
        # xq first (gates Q proj -> first attention chunk), then xk pieces in
        # consumption order; weights via gpsimd.
        for h in range(2):
            for cc in range(2):
                nc.sync.dma_start(
                    xq_t[cc][:, h * 1024 : (h + 1) * 1024],
                    xq_d[cc * 128 : (cc + 1) * 128, h * 1024 : (h + 1) * 1024],
                )
        for h in range(4):
            for cc in range(2):
                nc.sync.dma_start(
                    xk_t[cc][:, h * 1024 : (h + 1) * 1024],
                    xk_d[cc * 128 : (cc + 1) * 128, h * 1024 : (h + 1) * 1024],
                )

        def emit_proj_tile(pool, dst, wnm, xt, t):
            ps = pool.tile([128, 512], f32, tag="projps", name="ps")
            for cc in range(2):
                nc.tensor.matmul(
                    ps[:],
                    wts[(wnm, cc)][:],
                    xt[cc][:, t * 512 : (t + 1) * 512],
                    start=(cc == 0),
                    stop=(cc == 1),
                )
            nc.vector.tensor_copy(dst[:, t * 512 : (t + 1) * 512], ps[:])

        def emit_vt(pool, j):
            # V^T block j: [n-block, d] = x_block.T @ Wv.T
            tp = pool.tile([128, 512], f32, tag="projps", name="tp")
            for cc in range(2):
                nc.tensor.matmul(
                    tp[:, 0:128],
                    xk_t[cc][:, j * 128 : (j + 1) * 128],
                    wts[("wv", cc)][:],
                    start=(cc == 0),
                    stop=(cc == 1),
                )
            nc.vector.tensor_copy(VT[:, j * 128 : (j + 1) * 128], tp[:, 0:128])

        with tc.tile_pool(name="projps", bufs=4, space="PSUM") as pps:
            for t in range(MQ // 512):
                emit_proj_tile(pps, Q_t, "wq", xq_t, t)
            emit_proj_tile(pps, K_t, "wk", xk_t, 0)
            for j in range(2):
                emit_vt(pps, j)

        with (
            tc.tile_pool(name="spool", bufs=2, space="PSUM") as spool,
            tc.tile_pool(name="pcpool", bufs=1, space="PSUM") as pcpool,
            tc.tile_pool(name="lzps", bufs=2, space="PSUM") as lzps,
            tc.tile_pool(name="epool", bufs=6) as epool,
            tc.tile_pool(name="treep", bufs=2) as treep,
            tc.tile_pool(name="opool", bufs=2) as opool,
        ):
            pairs = [(qt, j) for qt in range(2) for j in range(KC)]
            ps_tiles = {}

            def emit_qk(qt, j):
                ps = spool.tile([128, QT], f32, tag="ps", name="ps")
                for qq in range(2):
                    nc.tensor.matmul(
                        ps[:, qq * 512 : (qq + 1) * 512],
                        K_t[:, j * 128 : (j + 1) * 128],
                        Q_t[:, qt * QT + qq * 512 : qt * QT + (qq + 1) * 512],
                        start=True,
                        stop=True,
                    )
                ps_tiles[(qt, j)] = ps

            # binary-counter pairwise reduction of exp tiles on DVE
            pending = []

            def tree_push(t, level=0):
                while pending and pending[-1][0] == level:
                    _, other = pending.pop()
                    nt = treep.tile(
                        [128, QT], bf16, tag=f"l{level + 1}", name=f"tl{level + 1}"
                    )
                    nc.vector.tensor_add(nt[:], other[:], t[:])
                    t, level = nt, level + 1
                pending.append((level, t))

            pc = None
            emit_qk(*pairs[0])
            for i, (qt, j) in enumerate(pairs):
                if i + 1 < len(pairs):
                    emit_qk(*pairs[i + 1])
                if 1 <= i <= 7:
                    emit_proj_tile(lzps, K_t, "wk", xk_t, i)   # K tile i for chunks 4i..
                if qt == 0 and j + 2 < KC:
                    emit_vt(lzps, j + 2)
                if j == 0:
                    pc = pcpool.tile([128, QT], f32, tag="pc", name="pc")
                ps = ps_tiles.pop((qt, j))
                es = epool.tile([128, QT], bf16, tag="es", name="es")
                nc.scalar.activation(es[:], ps[:], Exp)
                first, last = j == 0, j == KC - 1
                for qq in range(2):
                    sl = slice(qq * 512, (qq + 1) * 512)
                    nc.tensor.matmul(
                        pc[:, sl],
                        VT[:, j * 128 : (j + 1) * 128],
                        es[:, sl],
                        start=first,
                        stop=last,
                    )
                tree_push(es)
                if last:
                    acc = pending.pop()[1]
                    pending.clear()
                    so = opool.tile([128, QT], f32, tag="so", name="so")
                    nc.vector.tensor_copy(so[:], pc[:])
                    nc.sync.dma_start(oc_d[qt, :, :], so[:])
                    nc.sync.dma_start(oss_d[qt, :, :], acc[:])

    nc.compile()
    return nc


def make_in_maps(x, Wq, Wk, Wv):
    x = np.ascontiguousarray(np.asarray(x, dtype=np.float32).reshape(B, C, N))
    wt = {
        "wq": np.ascontiguousarray(np.asarray(Wq, dtype=np.float32).T),
        "wk": np.ascontiguousarray(np.asarray(Wk, dtype=np.float32).T),
        "wv": np.ascontiguousarray(np.asarray(Wv, dtype=np.float32).T),
    }

    in_maps = []
    for core in range(8):
        b, h = core // 2, core % 2
        in_maps.append(
            {
                "xk": x[b],
                "xq": np.ascontiguousarray(x[b][:, h * MQ : (h + 1) * MQ]),
                **wt,
            }
        )
    return in_maps


def assemble_output(results):
    out = np.empty((B, VC, N), dtype=np.float32)
    for core, r in enumerate(results):
        b, h = core // 2, core % 2
        sums = r["oss"].astype(np.float32).sum(axis=1, keepdims=True)  # [2,1,QT]
        core_out = r["oc"] / sums                                     # [2,128,QT]
        out[b, :, h * MQ : (h + 1) * MQ] = np.concatenate(
            [core_out[0], core_out[1]], axis=1
        )
    return out.reshape(B, VC, H, W)


def kernel(x, Wq, Wk, Wv):
    global _cached_nc
    from concourse.bass_utils import run_bass_kernel_spmd

    if _cached_nc is None:
        _cached_nc = _build()
    in_maps = make_in_maps(x, Wq, Wk, Wv)
    res = run_bass_kernel_spmd(_cached_nc, in_maps, core_ids=list(range(8)))
    return assemble_output(res.results)
